# revision 48
# baseline (speedup 1.0000x reference)
"""Multi-head attention kernel for 8 TRN2 NeuronCores.

Problem: b=2, n=2048, d=1024, heads=16, hd=64.
  q/k/v = x @ W{q,k,v}.T (+ zero bias)
  per head: softmax(q k^T / sqrt(d)) @ v
  out = concat @ Wo.T (+ zero bias)

Sharding (8 cores): data-parallel over batch (2) x tensor-parallel over
heads (16 heads -> 4 groups of 4). Core c handles batch c//4, heads
4*(c%4) .. 4*(c%4)+3 (feature slice of 256 columns). Wo is applied
row-parallel: each core emits a partial output; the host sums the 4
partials per batch (and untransposes). No collectives needed.

All matmuls run in float32r (TF32-like: ~1.5e-4 rel err on a K=1024
contraction, 4x the fp32 PE rate, full rate only when the moving free
dim is >=256). Raw fp32 bits are DMA'd directly into f32r tiles
(measured identical to an explicit rounding pass). On-chip f32r
operands (Q^T/K^T/V/P^T/out^T) are written by rounding copy/activation
producers as the walrus verifier requires.

Key structure decisions (all measured on HW):
 - everything is pre-transposed on the host so the kernel needs zero
   on-device transposes: xT (d,n), wqT/wkT/wvT (d,256), woT (256,d).
 - Q^T/K^T [feat, n] via dc-outer accumulation streaming behind the
   xT DMA; V in natural [n, feat] layout with a ones column appended
   (the ones column accumulates the softmax denominators during AV).
 - K^T is stored zero-padded per head to a full 128-row stationary:
   K=64 matmuls run at 2 cyc/row and read as low PE activity (HAM
   clock-gates to half speed); zero-padded K=128 runs at 1 cyc/row.
 - scores^T[k, q] (PE) -> exp via ScalarE reading 2 PSUM banks per
   call (the ACT engine is the pacing floor: n*n*heads/core exps at 1
   elem/cycle/lane) -> AV accumulates V_aug^T . P^T in PSUM [65, q].
 - passes are (q-half, head)-ordered and their emission is interleaved
   with the fc=1 projections so the PE fills ACT-paced slack; each
   q-half's output projection runs in the next half's ACT shadow.
 - normalize: copy avo out of PSUM fast (frees the accumulator), then
   reciprocal in a [128, 8] partition-scattered layout (a [1, 1024]
   row reciprocal is single-lane and 60x slower), partition_broadcast
   on GpSimd, multiply on DVE.
 - output projection keeps woT stationary (2 moving blocks per weight
   load) and emits the partial TRANSPOSED [d, n]; the host untransposes.

Biases are structurally zero in this problem spec and are skipped.
"""

import numpy as np

HEADS = 16
D = 1024
N = 2048
B = 2
N_CORES = 8
HPC = HEADS // (N_CORES // B)  # heads per core = 4
HD = D // HEADS                # 64
F = HPC * HD                   # 256 features per core
P = 128


def build_nc(n=N, d=D, hpc=HPC, hd=HD):
    """Build the per-core Bass program (SPMD: same program on all 8 cores)."""
    import concourse.bass as bass
    import concourse.tile as tile
    from concourse import bacc, mybir

    f32 = mybir.dt.float32
    f32r = mybir.dt.float32r
    f = hpc * hd            # per-core feature count (256)
    FC = f // P             # feature chunks (2)
    DC = d // P             # contraction chunks over d (8)
    NT = n // P             # n tiles / k chunks (16)
    QB = min(512, n)        # matmul moving block
    SCW = min(1024, n)      # scores psum width (2 banks)
    NSC = n // SCW          # q-halves
    scale = 1.0 / float(np.sqrt(np.float32(d)))

    nc = bacc.Bacc("TRN2")

    xT = nc.declare_dram_parameter("xT", [d, n], f32r, isOutput=False)
    wqT = nc.declare_dram_parameter("wqT", [d, f], f32r, isOutput=False)
    wkT = nc.declare_dram_parameter("wkT", [d, f], f32r, isOutput=False)
    wvT = nc.declare_dram_parameter("wvT", [d, f], f32r, isOutput=False)
    woT = nc.declare_dram_parameter("woT", [f, d], f32r, isOutput=False)
    out = nc.declare_dram_parameter("out", [d, n], f32, isOutput=True)

    xT_c = xT.rearrange("(c p) n -> c p n", p=P)
    wqT_c = wqT.rearrange("(c p) f -> c p f", p=P)
    wkT_c = wkT.rearrange("(c p) f -> c p f", p=P)
    wvT_c = wvT.rearrange("(c p) f -> c p f", p=P)
    woT_c = woT.rearrange("(c p) n -> c p n", p=P)

    with tile.TileContext(nc) as tc:
        with (
            tc.tile_pool(name="qkv", bufs=1) as qkv,
            tc.tile_pool(name="outT", bufs=1) as outp,
            # phase-2 pools created before the phase-1 pools so their
            # SBUF/PSUM ranges are disjoint: early heads' attention overlaps
            # the fc=1 projections with no pool-reuse serialization
            tc.tile_pool(name="pt", bufs=3) as ptp,
            tc.tile_pool(name="norm", bufs=1) as normp,
            tc.tile_pool(name="scps", bufs=2, space="PSUM") as scps,
            tc.tile_pool(name="avps", bufs=1, space="PSUM") as avps,
        ):
            QT_sb = qkv.tile([P, FC, n], f32r)
            # per-head K^T, zero-padded to a full 128-row stationary (head h
            # occupies partition rows po..po+hd, matching its rows in QT)
            KTz_sb = qkv.tile([P, hpc, n], f32r)
            V_sb = qkv.tile([P, NT, hpc, hd + 1], f32r)
            outT_sb = outp.tile([P, FC, n], f32r)
            # ones column of V_aug / zero fill of KTz: memset f32 consts, then
            # write via rounding DVE copies (direct memset on f32r fails
            # walrus codegen, and f32r matmul operands need rounding writers)
            ones_c = outp.tile([P, 1], f32)
            nc.vector.memset(ones_c[:], 1.0)
            nc.vector.tensor_copy(
                V_sb[:, :, :, hd : hd + 1],
                ones_c.to_broadcast([P, NT, hpc, 1]),
            )
            zero_c = outp.tile([P, 1], f32)
            nc.vector.memset(zero_c[:], 0.0)
            nc.vector.tensor_copy(
                KTz_sb[:], zero_c.to_broadcast([P, hpc, n])
            )
            # dummy exp to pull the ~2.7us ACT table load into the DMA phase
            warm = outp.tile([P, 1], f32)
            nc.scalar.activation(
                warm[:], zero_c[:], mybir.ActivationFunctionType.Exp
            )

            def pass_begin():
                return avps.tile([hd + 1, SCW], f32, tag="avo", name="avo")

            def pass_blocks(avo, h, sh, kcs, pre_kc=None):
                """scores^T -> exp -> AV accumulate for k-chunks `kcs`."""
                fc = (h * hd) // P
                q0 = sh * SCW
                for kc in kcs:
                    if pre_kc is not None:
                        pre_kc(kc)
                    sc = scps.tile([P, SCW], f32, tag="sc")
                    for qc in range(SCW // QB):
                        nc.tensor.matmul(
                            sc[:, qc * QB : (qc + 1) * QB],
                            KTz_sb[:, h, kc * P : (kc + 1) * P],
                            QT_sb[:, fc, q0 + qc * QB : q0 + (qc + 1) * QB],
                            start=True,
                            stop=True,
                        )
                    pt = ptp.tile([P, SCW], f32r, tag="pt")
                    nc.scalar.activation(
                        pt[:], sc[:], mybir.ActivationFunctionType.Exp,
                        scale=scale,
                    )
                    for qc in range(SCW // QB):
                        nc.tensor.matmul(
                            avo[:, qc * QB : (qc + 1) * QB],
                            V_sb[:, kc, h, :],
                            pt[:, qc * QB : (qc + 1) * QB],
                            start=(kc == 0),
                            stop=(kc == NT - 1),
                        )

            def pass_end(avo, h, sh):
                """Free avo fast, then normalize rows 0..hd-1 by row hd (the
                softmax sums). reciprocal is single-lane-slow on a [1, SCW]
                row, so scatter the sums across partitions via a small SBUF
                DMA round-trip first."""
                fc = (h * hd) // P
                po = (h * hd) % P
                q0 = sh * SCW
                av_sb = normp.tile([hd + 1, SCW], f32, tag="av_sb")
                nc.vector.tensor_copy(av_sb[:], avo[:])
                rsh = normp.tile([P, SCW // P], f32, tag="rsh")
                nc.sync.dma_start(out=rsh[:], in_=av_sb[hd : hd + 1, :])
                rsh2 = normp.tile([P, SCW // P], f32, tag="rsh2")
                nc.vector.reciprocal(rsh2[:], rsh[:])
                recip = normp.tile([1, SCW], f32, tag="recip")
                nc.sync.dma_start(out=recip[:], in_=rsh2[:])
                bc = normp.tile([hd, SCW], f32, tag="bc")
                nc.gpsimd.partition_broadcast(bc[:], recip[:])
                nc.vector.tensor_mul(
                    outT_sb[po : po + hd, fc, q0 : q0 + SCW],
                    av_sb[0:hd, :],
                    bc[:],
                )

            def do_pass(h, sh, pre_kc=None):
                avo = pass_begin()
                pass_blocks(avo, h, sh, range(NT), pre_kc=pre_kc)
                pass_end(avo, h, sh)

            # ---- Phase 1 + first q-half heads 0/1, emission-interleaved ----
            with (
                tc.tile_pool(name="xw", bufs=1) as xw,
                tc.tile_pool(name="p1ps", bufs=2, space="PSUM") as p1ps,
            ):
                xT_r = xw.tile([P, DC, n], f32r)
                wqT_r = xw.tile([P, DC, f], f32r)
                wkT_r = xw.tile([P, DC, f], f32r)
                wvT_r = xw.tile([P, DC, f], f32r)

                # wq + xT interleaved per chunk: QT matmuls stream right
                # behind them; wk/wv stream during QT/KT compute.
                for dc in range(DC):
                    nc.sync.dma_start(out=wqT_r[:, dc, :], in_=wqT_c[dc])
                    nc.sync.dma_start(out=xT_r[:, dc, :], in_=xT_c[dc])

                def proj_cols(w_sb, is_k, fc, qcp):
                    # dc-outer accumulation, one sub-stage of 2 held banks
                    # covering moving columns [qcp*QB, (qcp+2)*QB)
                    pss = [
                        p1ps.tile([P, QB], f32, tag="big", name=f"pj{g}")
                        for g in range(2)
                    ]
                    for dc in range(DC):
                        for j in range(2):
                            qc = qcp + j
                            nc.tensor.matmul(
                                pss[j][:],
                                w_sb[:, dc, fc * P : (fc + 1) * P],
                                xT_r[:, dc, qc * QB : (qc + 1) * QB],
                                start=(dc == 0),
                                stop=(dc == DC - 1),
                            )
                    for j in range(2):
                        qc = qcp + j
                        sl = slice(qc * QB, (qc + 1) * QB)
                        if is_k:
                            # rows 0:64 = head 2fc (po=0), rows 64:128 =
                            # head 2fc+1 (po=64); keep row alignment
                            nc.vector.tensor_copy(
                                KTz_sb[0:hd, 2 * fc, sl], pss[j][0:hd, :]
                            )
                            nc.vector.tensor_copy(
                                KTz_sb[hd : 2 * hd, 2 * fc + 1, sl],
                                pss[j][hd : 2 * hd, :],
                            )
                        else:
                            nc.vector.tensor_copy(QT_sb[:, fc, sl], pss[j][:])

                def v_tile(nt):
                    ps = p1ps.tile([P, QB], f32, tag="big", name="vps")
                    for dc in range(DC):
                        nc.tensor.matmul(
                            ps[:, 0:f],
                            xT_r[:, dc, nt * P : (nt + 1) * P],
                            wvT_r[:, dc, :],
                            start=(dc == 0),
                            stop=(dc == DC - 1),
                        )
                    nc.vector.tensor_copy(
                        V_sb[:, nt, :, 0:hd],
                        ps[:, 0:f].rearrange("p (h e) -> p h e", h=hpc),
                    )

                # wk needed right after the first k0 sub-stage; wv by the
                # first v_tile — both AFTER the xT stream in queue order so
                # they don't delay the projection-gating xT chunks
                for dc in range(DC):
                    nc.sync.dma_start(out=wkT_r[:, dc, :], in_=wkT_c[dc])
                for dc in range(DC):
                    nc.sync.dma_start(out=wvT_r[:, dc, :], in_=wvT_c[dc])
                # Emission order = scheduling priority. Minimal chain to the
                # first exp: QT cols of the first q-half, then K^T in column
                # sub-stages interleaved with head 0's pass blocks (V tiles
                # interleaved per k-chunk they feed). Later projections are
                # emitted after the passes they should yield priority to, so
                # they fill the PE's ACT-paced slack.
                proj_cols(wqT_r, False, 0, 0)  # QT fc0 cols 0:1024 (q-half 0)
                avo0 = pass_begin()
                proj_cols(wkT_r, True, 0, 0)   # KTz fc0 cols 0:1024 (kc 0..7)
                pass_blocks(avo0, 0, 0, range(0, NT // 2), pre_kc=v_tile)
                proj_cols(wkT_r, True, 0, 2)   # KTz fc0 cols 1024:2048
                pass_blocks(avo0, 0, 0, range(NT // 2, NT), pre_kc=v_tile)
                pass_end(avo0, 0, 0)
                do_pass(1, 0)
                proj_cols(wqT_r, False, 0, 2)  # QT fc0 cols for q-half 1
                do_pass(0, 1)
                do_pass(1, 1)
                proj_cols(wqT_r, False, 1, 0)
                proj_cols(wqT_r, False, 1, 2)
                proj_cols(wkT_r, True, 1, 0)
                proj_cols(wkT_r, True, 1, 2)

            # ---- remaining passes + per-q-half output projection ----
            with (
                tc.tile_pool(name="wo", bufs=1) as wop,
                tc.tile_pool(name="wops", bufs=2, space="PSUM") as wopsp,
                tc.tile_pool(name="wosb", bufs=4) as wosbp,
            ):
                woT_sb = wop.tile([P, FC, d], f32r)
                for fc in range(FC):
                    nc.sync.dma_start(out=woT_sb[:, fc, :], in_=woT_c[fc])

                def wo_half(sh):
                    # output projection for q-half sh (woT stationary, 2
                    # moving q-blocks per weight load; emits partial^T [d, n])
                    q0 = sh * SCW
                    for do in range(d // P):
                        pss = [
                            wopsp.tile([P, QB], f32, tag="wops", name=f"wo{i}")
                            for i in range(SCW // QB)
                        ]
                        for fc in range(FC):
                            for qc in range(SCW // QB):
                                nc.tensor.matmul(
                                    pss[qc][:],
                                    woT_sb[:, fc, do * P : (do + 1) * P],
                                    outT_sb[
                                        :, fc, q0 + qc * QB : q0 + (qc + 1) * QB
                                    ],
                                    start=(fc == 0),
                                    stop=(fc == FC - 1),
                                )
                        for qc in range(SCW // QB):
                            ob = wosbp.tile([P, QB], f32, tag="ob")
                            nc.vector.tensor_copy(ob[:], pss[qc][:])
                            nc.sync.dma_start(
                                out=out[
                                    do * P : (do + 1) * P,
                                    q0 + qc * QB : q0 + (qc + 1) * QB,
                                ],
                                in_=ob[:],
                            )

                do_pass(2, 0)
                do_pass(3, 0)
                wo_half(0)
                do_pass(2, 1)
                do_pass(3, 1)
                wo_half(1)
    nc.finalize()
    return nc


def make_in_maps(x, Wq, Wk, Wv, Wo):
    """Shard full inputs into per-core DRAM parameter maps."""
    x = np.asarray(x, dtype=np.float32)
    Wq = np.asarray(Wq, dtype=np.float32)
    Wk = np.asarray(Wk, dtype=np.float32)
    Wv = np.asarray(Wv, dtype=np.float32)
    Wo = np.asarray(Wo, dtype=np.float32)
    xTs = [np.ascontiguousarray(x[b].T) for b in range(B)]
    WqT, WkT, WvT = Wq.T, Wk.T, Wv.T
    in_maps = []
    for c in range(N_CORES):
        b, g = c // (N_CORES // B), c % (N_CORES // B)
        fs = slice(g * F, (g + 1) * F)
        in_maps.append(
            {
                "xT": xTs[b],
                "wqT": np.ascontiguousarray(WqT[:, fs]),
                "wkT": np.ascontiguousarray(WkT[:, fs]),
                "wvT": np.ascontiguousarray(WvT[:, fs]),
                "woT": np.ascontiguousarray(Wo[:, fs].T),
            }
        )
    return in_maps


_NC_CACHE = {}


def _enable_ldw_opt():
    """Flip walrus --enable-ldw-opt to true: consecutive matmuls sharing a
    stationary operand skip the redundant LDWEIGHTS reload."""
    import concourse.bass_utils as bu

    if getattr(bu, "_ldw_opt_patched", False):
        return
    orig = bu.run_command

    def patched(argv, **kw):
        argv = [
            "--enable-ldw-opt=true" if a == "--enable-ldw-opt=false" else a
            for a in argv
        ]
        return orig(argv, **kw)

    bu.run_command = patched
    bu._ldw_opt_patched = True


def run(x, Wq, Wk, Wv, Wo, trace=False):
    from concourse.bass_utils import run_bass_kernel_spmd

    _enable_ldw_opt()
    if "nc" not in _NC_CACHE:
        _NC_CACHE["nc"] = build_nc()
    nc = _NC_CACHE["nc"]
    in_maps = make_in_maps(x, Wq, Wk, Wv, Wo)
    res = run_bass_kernel_spmd(nc, in_maps, core_ids=list(range(N_CORES)), trace=trace)
    parts = [np.asarray(res.results[i]["out"]) for i in range(N_CORES)]
    gpb = N_CORES // B
    # per-core partials are transposed [d, n]: sum the group, then untranspose
    full = np.stack(
        [
            sum(parts[b * gpb + 1 : (b + 1) * gpb], parts[b * gpb]).T
            for b in range(B)
        ]
    )
    return np.ascontiguousarray(full, dtype=np.float32), res


def kernel(x, Wq, bq, Wk, bk, Wv, bv, Wo, bo):
    full, _ = run(x, Wq, Wk, Wv, Wo)
    return full
